# revision 1
# baseline (speedup 1.0000x reference)
"""TRN2 Bass kernel for nn_FP8LinearWrapper: y = x @ (w_fp8 * inv_scale).T + bias.

Strategy (8 NeuronCores, SPMD):
  - Data-parallel over the flattened token dim: x [4,2048,4096] -> [8192,4096],
    1024 rows per core. Weights/bias replicated to every core.
  - Per core: dual-pass bf16 matmul (x split into bf16 hi + bf16 lo parts on
    device) against the exactly-dequantized bf16 weight, accumulating both
    passes in fp32 PSUM. Result is fp32-quality (~3e-6 rel err).
  - The fp8 weight bytes are jax float8_e4m3fn (max 448). TRN2's fp8e4 decode
    is IEEE e4m3 (max 240), so the host re-encodes each byte via a LUT to the
    e4m3 bits of (value/2) - exact for all normals - and the kernel folds the
    missing *2 into the output scale. Weights stay 1 byte; all arithmetic
    (dequant cast, transpose of x, matmul, scale, bias) runs on device.
  - x is transposed on device via PE-transpose (contraction dim must be on
    SBUF partitions for both matmul operands). w is passed pre-transposed /
    pre-blocked (weight layout prep, as for any serving stack).

Timing-critical structure (~958 us/core, PE 95% busy at the 216 ns/MM
N=512 bf16 streaming roofline):
  - Phase A: per m-tile, x loads interleave with PE-transposes (8 per chunk)
    and o-block-0 matmul bursts (16 per chunk) so the HAM clock gate stays at
    8/8; lo parts are consumed from SBUF for o-block 0 and spilled to DRAM
    for the rest.
  - O-blocks 1..7: m-tile PAIRS share one 2-bank PSUM tile (2 x 64
    accumulating matmuls, 32 k-tiles x {hi,lo} each) with a single fused
    (psum * 2*inv_scale) + bias DVE eviction covering both banks, halving
    group-boundary syncs.
Remaining non-compute time is Tile-framework fixed cost (launch preamble
~12us, drain barrier ~11us, per-group event-semaphore beats ~18us).
"""

import os
import sys

for _p in (
    "/opt/trn_rl_repo",
    "/root/.axon_site",
    "/root/.axon_site/_ro/trn_rl_repo",
    "/root/.axon_site/_ro/pypackages",
):
    if os.path.isdir(_p) and _p not in sys.path:
        sys.path.append(_p)

import numpy as np
import ml_dtypes

B, S, DI, DO = 4, 2048, 4096, 4096
NCORES = 8
M = B * S            # 8192
MC = M // NCORES     # 1024 rows per core
P = 128
KT = DI // P         # 32 k-tiles
MT = MC // P         # 8 m-tiles per core
OBW = 512            # o-block width
OB = DO // OBW       # 8 o-blocks
WCK = 4              # k-tiles per weight chunk
WCH = KT // WCK      # 8 weight chunks per o-block

_STATE = {}


def _build_program():
    import concourse.bass as bass
    import concourse.mybir as mybir
    import concourse.tile as tile
    from concourse import bacc
    from concourse.masks import make_identity

    dt = mybir.dt
    F32, BF16, FP8 = dt.float32, dt.bfloat16, dt.float8e4

    nc = bacc.Bacc(target_bir_lowering=False)

    x_in = nc.dram_tensor("x", [MC, DI], F32, kind="ExternalInput")
    w_in = nc.dram_tensor("w", [OB, P, KT, OBW], FP8, kind="ExternalInput")
    s_in = nc.dram_tensor("s", [P, 1], F32, kind="ExternalInput")
    b_in = nc.dram_tensor("b", [P, DO], F32, kind="ExternalInput")
    y_out = nc.dram_tensor("y", [MC, DO], F32, kind="ExternalOutput")
    lo_dram = nc.dram_tensor("lo_scratch", [MT, P, KT, P], BF16)

    with tile.TileContext(nc) as tc:
        with (
            tc.tile_pool(name="const", bufs=1) as const,
            tc.tile_pool(name="xt_hi_pool", bufs=1) as xt_hi_pool,
            tc.tile_pool(name="xin_pool", bufs=3) as xin_pool,
            tc.tile_pool(name="lo_asm_pool", bufs=3) as lo_asm_pool,
            tc.tile_pool(name="lo_work_pool", bufs=3) as lo_work_pool,
            tc.tile_pool(name="w8_pool", bufs=10) as w8_pool,
            tc.tile_pool(name="wb_pool", bufs=10) as wb_pool,
            tc.tile_pool(name="bias_pool", bufs=2) as bias_pool,
            tc.tile_pool(name="out_pool", bufs=2) as out_pool,
            tc.tile_pool(name="tp_ps_pool", bufs=4, space="PSUM") as tp_ps_pool,
            tc.tile_pool(name="mm_ps_pool", bufs=2, space="PSUM") as mm_ps_pool,
        ):
            ident = const.tile([P, P], F32)
            make_identity(nc, ident)
            s_t = const.tile([P, 1], F32)
            nc.sync.dma_start(out=s_t, in_=s_in[:, :])
            s2 = const.tile([P, 1], F32)
            nc.scalar.mul(s2, s_t, 2.0)  # fold back the /2 from the fp8 re-encode

            # resident transposed hi part of x: [128 d, 32 kt, 1024 m] bf16
            xt_hi = xt_hi_pool.tile([P, KT, MC], BF16)

            def load_wchunks(ob):
                bias_sl = bias_pool.tile([P, OBW], F32, name=f"bias_{ob}", tag="bias")
                nc.sync.dma_start(out=bias_sl, in_=b_in[:, ob * OBW:(ob + 1) * OBW])
                wchunks = []
                for c in range(WCH):
                    w8c = w8_pool.tile([P, WCK, OBW], FP8, name=f"w8_{ob}_{c}", tag="w8")
                    nc.sync.dma_start(out=w8c, in_=w_in[ob, :, c * WCK:(c + 1) * WCK, :])
                    wbc = wb_pool.tile([P, WCK, OBW], BF16, name=f"wb_{ob}_{c}", tag="wb")
                    nc.scalar.copy(wbc, w8c)
                    wchunks.append(wbc)
                return bias_sl, wchunks

            def emit_mms(ps_slice, ob, mt, lo_tile, wchunks, skip_check=False):
                for kt in range(KT):
                    wb_sl = wchunks[kt // WCK][:, kt % WCK, :]
                    nc.tensor.matmul(
                        ps_slice, xt_hi[:, kt, mt * P:(mt + 1) * P], wb_sl,
                        start=(kt == 0), stop=False, skip_group_check=skip_check,
                    )
                    nc.tensor.matmul(
                        ps_slice, lo_tile[:, kt, :], wb_sl,
                        start=False, stop=(kt == KT - 1), skip_group_check=skip_check,
                    )

            def mm_group_pair(ob, mt0, lo_a, lo_b, bias2, wchunks):
                # two m-tile groups share one 2-bank PSUM tile and a single
                # fused eviction -> half the group-boundary syncs on PE
                ps = mm_ps_pool.tile([P, 2 * OBW], F32, name=f"ps_{ob}_{mt0}", tag="ps")
                emit_mms(ps[:, 0:OBW], ob, mt0, lo_a, wchunks, skip_check=True)
                emit_mms(ps[:, OBW:2 * OBW], ob, mt0 + 1, lo_b, wchunks, skip_check=True)
                out_sb = out_pool.tile([P, 2 * OBW], F32, name=f"o_{ob}_{mt0}", tag="out")
                nc.vector.scalar_tensor_tensor(
                    out_sb, ps, s2[:, :], bias2,
                    mybir.AluOpType.mult, mybir.AluOpType.add,
                )
                for h, mt in ((0, mt0), (1, mt0 + 1)):
                    nc.sync.dma_start(
                        out=y_out[mt * P:(mt + 1) * P, ob * OBW:(ob + 1) * OBW],
                        in_=out_sb[:, h * OBW:(h + 1) * OBW],
                    )

            # ---- Phase A: PE-transpose x into hi (resident SBUF) + lo, finely
            # interleaved with o-block-0 matmuls (8 transposes -> 16 matmuls per
            # 1024-col chunk) so the matmul bursts keep HAM at full clock and
            # the open PSUM group hides transpose latency. lo is read straight
            # from SBUF for o-block 0. ----
            first_xins = []
            for c in range(4):  # mt0's loads beat the 16MB weight prefetch
                xin = xin_pool.tile([P, 1024], F32, name=f"xin_0_{c}", tag="xin")
                nc.sync.dma_start(out=xin, in_=x_in[0:P, c * 1024:(c + 1) * 1024])
                first_xins.append(xin)
            bias0, wch0 = load_wchunks(0)
            for mt in range(MT):
                lo_asm = lo_asm_pool.tile([P, KT, P], BF16, name=f"lo_{mt}", tag="lo")
                ps = mm_ps_pool.tile([P, OBW], F32, name=f"ps_0_{mt}", tag="ps")
                for c in range(4):
                    if mt == 0:
                        xin = first_xins[c]
                    else:
                        xin = xin_pool.tile([P, 1024], F32, name=f"xin_{mt}_{c}", tag="xin")
                        nc.sync.dma_start(
                            out=xin,
                            in_=x_in[mt * P:(mt + 1) * P, c * 1024:(c + 1) * 1024],
                        )
                    for kk in range(8):
                        kt = c * 8 + kk
                        tp = tp_ps_pool.tile([P, P], F32, name=f"tp_{mt}_{kt}", tag="tp")
                        nc.tensor.matmul(
                            tp, xin[:, kk * P:(kk + 1) * P], ident,
                            is_transpose=True, skip_group_check=True,
                        )
                        hi_sl = xt_hi[:, kt, mt * P:(mt + 1) * P]
                        nc.scalar.copy(hi_sl, tp)
                        nc.vector.tensor_sub(lo_asm[:, kt, :], tp, hi_sl)
                    for kk in range(8):
                        kt = c * 8 + kk
                        wb_sl = wch0[kt // WCK][:, kt % WCK, :]
                        nc.tensor.matmul(
                            ps, xt_hi[:, kt, mt * P:(mt + 1) * P], wb_sl,
                            start=(kt == 0), stop=False, skip_group_check=True,
                        )
                        nc.tensor.matmul(
                            ps, lo_asm[:, kt, :], wb_sl,
                            start=False, stop=(kt == KT - 1), skip_group_check=True,
                        )
                nc.sync.dma_start(out=lo_dram[mt], in_=lo_asm[:, :, :])
                out_sb = out_pool.tile([P, OBW], F32, name=f"o_0_{mt}", tag="out")
                nc.vector.scalar_tensor_tensor(
                    out_sb, ps, s2[:, :], bias0,
                    mybir.AluOpType.mult, mybir.AluOpType.add,
                )
                nc.sync.dma_start(
                    out=y_out[mt * P:(mt + 1) * P, 0:OBW], in_=out_sb,
                )

            # ---- Phase B: remaining o-blocks, lo streamed back from DRAM,
            # m-tiles processed in pairs sharing one 2-bank PSUM tile ----
            for ob in range(1, OB):
                bias_sl, wchunks = load_wchunks(ob)
                bias2 = bias_pool.tile([P, 2 * OBW], F32, name=f"bias2_{ob}", tag="bias2")
                for h in range(2):
                    nc.sync.dma_start(
                        out=bias2[:, h * OBW:(h + 1) * OBW],
                        in_=b_in[:, ob * OBW:(ob + 1) * OBW],
                    )
                for mt0 in range(0, MT, 2):
                    lo_a = lo_work_pool.tile([P, KT, P], BF16, name=f"low_{ob}_{mt0}", tag="low")
                    nc.sync.dma_start(out=lo_a, in_=lo_dram[mt0])
                    lo_b = lo_work_pool.tile([P, KT, P], BF16, name=f"low_{ob}_{mt0 + 1}", tag="low")
                    nc.sync.dma_start(out=lo_b, in_=lo_dram[mt0 + 1])
                    mm_group_pair(ob, mt0, lo_a, lo_b, bias2, wchunks)

    nc.finalize()
    return nc


def _get_program():
    if "nc" not in _STATE:
        _STATE["nc"] = _build_program()
    return _STATE["nc"]


def _prep_weights(weight_fp8):
    """Re-encode jax e4m3fn bytes as IEEE-e4m3 bytes of value/2 (exact for
    normals), transpose to [d, o], and block to [ob, p, kt, obw] so each
    o-block DMA reads 2KB-contiguous per-partition lines."""
    bits = np.arange(256, dtype=np.uint8)
    vals = bits.view(ml_dtypes.float8_e4m3fn).astype(np.float32) * 0.5
    lut = vals.astype(ml_dtypes.float8_e4m3).view(np.uint8)

    wb = np.asarray(weight_fp8).view(np.uint8)          # [DO, DI]
    w2t = np.ascontiguousarray(lut[wb].T)               # [DI, DO]
    w_pre = np.ascontiguousarray(
        w2t.reshape(KT, P, OB, OBW).transpose(2, 1, 0, 3)
    )                                                   # [OB, P, KT, OBW]
    return w_pre.view(ml_dtypes.float8_e4m3)


def kernel(x, weight_fp8, weight_inv_scale, bias):
    from concourse.bass_utils import run_bass_kernel_spmd

    try:
        import jax
        jax.config.update("jax_compilation_cache_dir", "/tmp/jax_neff_cache")
        jax.config.update("jax_persistent_cache_min_entry_size_bytes", 0)
        jax.config.update("jax_persistent_cache_min_compile_time_secs", 0.0)
    except Exception:
        pass

    nc = _get_program()

    x_np = np.asarray(x, dtype=np.float32).reshape(M, DI)
    w_pre = _prep_weights(weight_fp8)
    s_b = np.ascontiguousarray(
        np.broadcast_to(
            np.asarray(weight_inv_scale, dtype=np.float32).reshape(1, 1), (P, 1)
        )
    )
    b_b = np.ascontiguousarray(
        np.broadcast_to(np.asarray(bias, dtype=np.float32), (P, DO))
    )

    core_ids = list(range(NCORES))
    in_maps = [
        {"x": x_np[c * MC:(c + 1) * MC], "w": w_pre, "s": s_b, "b": b_b}
        for c in core_ids
    ]

    last_err = None
    for _attempt in range(3):
        try:
            res = run_bass_kernel_spmd(nc, in_maps, core_ids)
            break
        except Exception as e:  # device wedge (NRT_EXEC_UNIT_UNRECOVERABLE): reset + retry
            last_err = e
            try:
                import jax
                import time
                jax.clear_backends()
                time.sleep(3.0)
            except Exception:
                pass
    else:
        raise last_err

    y = np.concatenate([res.results[c]["y"] for c in core_ids], axis=0)
    return y.reshape(B, S, DO)



# revision 2
# speedup vs baseline: 1.8901x; 1.8901x over previous
"""TRN2 Bass kernel for nn_FP8LinearWrapper: y = x @ (w_fp8 * inv_scale).T + bias.

Strategy (8 NeuronCores, SPMD):
  - Data-parallel over the flattened token dim: x [4,2048,4096] -> [8192,4096],
    1024 rows per core. Weights/bias replicated to every core.
  - Per core: SINGLE-pass bf16 matmul. x is rounded to bf16 on device (the
    PE-transpose + cast that produces the operand layout), the fp8 weight is
    fed directly as the matmul moving operand (mixed bf16 x fp8e4 matmul,
    verified bit-exact on HW), accumulation in fp32 PSUM. Error is dominated
    by the bf16 rounding of x: ~1e-3 rel absmax vs the 2e-2 gate (the
    dual-pass hi+lo scheme this replaces was 2x the PE work for accuracy the
    gate does not need).
  - The fp8 weight bytes are jax float8_e4m3fn (max 448). TRN2's fp8e4 decode
    is IEEE e4m3 (max 240), so the host re-encodes each byte via a LUT to the
    e4m3 bits of (value/2) - exact for all normals - and the kernel folds the
    missing *2 into the output scale. w is passed pre-transposed/pre-blocked
    (weight layout prep, as for any serving stack).
  - x is transposed on device via PE-transpose (contraction dim must be on
    SBUF partitions for both matmul operands); the transposed bf16 x (8MB)
    stays resident in SBUF and is reused by all 8 o-blocks.

Timing structure (~470 us/core of PE work at the 216 ns/MM N=512 streaming
roofline):
  - Phase T (o-block 0): m-tile PAIRS share one 2-bank PSUM tile; per m-tile,
    x loads interleave with PE-transposes (8 per 1024-col chunk) and o-block-0
    matmuls (8 per chunk) so the PE stays dense while x streams in.
  - O-blocks 1..7: m-tile pairs again share a 2-bank PSUM tile (2 x 32
    accumulating matmuls) with a single fused (psum * 2*inv_scale) + bias DVE
    eviction covering both banks, halving group-boundary syncs.
"""

import os
import sys

for _p in (
    "/opt/trn_rl_repo",
    "/root/.axon_site",
    "/root/.axon_site/_ro/trn_rl_repo",
    "/root/.axon_site/_ro/pypackages",
):
    if os.path.isdir(_p) and _p not in sys.path:
        sys.path.append(_p)

import numpy as np
import ml_dtypes

B, S, DI, DO = 4, 2048, 4096, 4096
NCORES = 8
M = B * S            # 8192
MC = M // NCORES     # 1024 rows per core
P = 128
KT = DI // P         # 32 k-tiles
MT = MC // P         # 8 m-tiles per core
OBW = 512            # o-block width
OB = DO // OBW       # 8 o-blocks
WCK = 4              # k-tiles per weight chunk
WCH = KT // WCK      # 8 weight chunks per o-block

_STATE = {}


def _build_program():
    import concourse.bass as bass
    import concourse.mybir as mybir
    import concourse.tile as tile
    from concourse import bacc
    from concourse.masks import make_identity

    dt = mybir.dt
    F32, BF16, FP8 = dt.float32, dt.bfloat16, dt.float8e4

    nc = bacc.Bacc(target_bir_lowering=False)

    x_in = nc.dram_tensor("x", [MC, DI], F32, kind="ExternalInput")
    w_in = nc.dram_tensor("w", [OB, P, KT, OBW], FP8, kind="ExternalInput")
    s_in = nc.dram_tensor("s", [P, 1], F32, kind="ExternalInput")
    b_in = nc.dram_tensor("b", [P, DO], F32, kind="ExternalInput")
    y_out = nc.dram_tensor("y", [MC, DO], F32, kind="ExternalOutput")

    with tile.TileContext(nc) as tc:
        with (
            tc.tile_pool(name="const", bufs=1) as const,
            tc.tile_pool(name="xt_pool", bufs=1) as xt_pool,
            tc.tile_pool(name="xin_pool", bufs=3) as xin_pool,
            tc.tile_pool(name="w8_pool", bufs=12) as w8_pool,
            tc.tile_pool(name="bias_pool", bufs=2) as bias_pool,
            tc.tile_pool(name="out_pool", bufs=2) as out_pool,
            tc.tile_pool(name="tp_ps_pool", bufs=4, space="PSUM") as tp_ps_pool,
            tc.tile_pool(name="mm_ps_pool", bufs=2, space="PSUM") as mm_ps_pool,
        ):
            ident = const.tile([P, P], F32)
            make_identity(nc, ident)
            s_t = const.tile([P, 1], F32)
            nc.sync.dma_start(out=s_t, in_=s_in[:, :])
            s2 = const.tile([P, 1], F32)
            nc.scalar.mul(s2, s_t, 2.0)  # fold back the /2 from the fp8 re-encode

            # resident transposed bf16 x: [128 d, 32 kt, 1024 m]
            xt = xt_pool.tile([P, KT, MC], BF16)

            def load_wchunks(ob):
                bias2 = bias_pool.tile([P, 2 * OBW], F32, name=f"bias2_{ob}", tag="bias")
                for h in range(2):
                    nc.sync.dma_start(
                        out=bias2[:, h * OBW:(h + 1) * OBW],
                        in_=b_in[:, ob * OBW:(ob + 1) * OBW],
                    )
                wchunks = []
                for c in range(WCH):
                    w8c = w8_pool.tile([P, WCK, OBW], FP8, name=f"w8_{ob}_{c}", tag="w8")
                    nc.sync.dma_start(out=w8c, in_=w_in[ob, :, c * WCK:(c + 1) * WCK, :])
                    wchunks.append(w8c)
                return bias2, wchunks

            def evict_pair(ps, ob, mt0, bias2):
                # fused (psum * s2) + bias over both halves, then 2 y DMAs
                out_sb = out_pool.tile([P, 2 * OBW], F32, name=f"o_{ob}_{mt0}", tag="out")
                nc.vector.scalar_tensor_tensor(
                    out_sb, ps, s2[:, :], bias2,
                    mybir.AluOpType.mult, mybir.AluOpType.add,
                )
                for h, mt in ((0, mt0), (1, mt0 + 1)):
                    nc.sync.dma_start(
                        out=y_out[mt * P:(mt + 1) * P, ob * OBW:(ob + 1) * OBW],
                        in_=out_sb[:, h * OBW:(h + 1) * OBW],
                    )

            # ---- Phase T: PE-transpose x into resident bf16 xt, finely
            # interleaved with o-block-0 matmuls (8 transposes -> 8 matmuls per
            # 1024-col chunk). m-tile pairs share one 2-bank PSUM tile with a
            # single fused eviction. ----
            first_xins = []
            for c in range(4):  # mt0's loads beat the 2MB o-block-0 w prefetch
                xin = xin_pool.tile([P, 1024], F32, name=f"xin_0_{c}", tag="xin")
                nc.sync.dma_start(out=xin, in_=x_in[0:P, c * 1024:(c + 1) * 1024])
                first_xins.append(xin)
            bias0, wch0 = load_wchunks(0)
            for mt0 in range(0, MT, 2):
                ps = mm_ps_pool.tile([P, 2 * OBW], F32, name=f"ps_0_{mt0}", tag="ps")
                for h, mt in ((0, mt0), (1, mt0 + 1)):
                    ps_h = ps[:, h * OBW:(h + 1) * OBW]
                    for c in range(4):
                        if mt == 0:
                            xin = first_xins[c]
                        else:
                            xin = xin_pool.tile([P, 1024], F32, name=f"xin_{mt}_{c}", tag="xin")
                            nc.sync.dma_start(
                                out=xin,
                                in_=x_in[mt * P:(mt + 1) * P, c * 1024:(c + 1) * 1024],
                            )
                        for kk in range(8):
                            kt = c * 8 + kk
                            tp = tp_ps_pool.tile([P, P], F32, name=f"tp_{mt}_{kt}", tag="tp")
                            nc.tensor.matmul(
                                tp, xin[:, kk * P:(kk + 1) * P], ident,
                                is_transpose=True, skip_group_check=True,
                            )
                            nc.scalar.copy(xt[:, kt, mt * P:(mt + 1) * P], tp)
                        for kk in range(8):
                            kt = c * 8 + kk
                            wb_sl = wch0[kt // WCK][:, kt % WCK, :]
                            nc.tensor.matmul(
                                ps_h, xt[:, kt, mt * P:(mt + 1) * P], wb_sl,
                                start=(kt == 0), stop=(kt == KT - 1),
                                skip_group_check=True,
                            )
                evict_pair(ps, 0, mt0, bias0)

            # ---- Phase B: o-blocks 1..7 stream w fp8 from DRAM against the
            # resident xt; m-tile pairs share one 2-bank PSUM tile ----
            for ob in range(1, OB):
                bias2, wchunks = load_wchunks(ob)
                for mt0 in range(0, MT, 2):
                    ps = mm_ps_pool.tile([P, 2 * OBW], F32, name=f"ps_{ob}_{mt0}", tag="ps")
                    for h, mt in ((0, mt0), (1, mt0 + 1)):
                        ps_h = ps[:, h * OBW:(h + 1) * OBW]
                        for kt in range(KT):
                            wb_sl = wchunks[kt // WCK][:, kt % WCK, :]
                            nc.tensor.matmul(
                                ps_h, xt[:, kt, mt * P:(mt + 1) * P], wb_sl,
                                start=(kt == 0), stop=(kt == KT - 1),
                                skip_group_check=True,
                            )
                    evict_pair(ps, ob, mt0, bias2)

    nc.finalize()
    return nc


def _get_program():
    if "nc" not in _STATE:
        _STATE["nc"] = _build_program()
    return _STATE["nc"]


def _prep_weights(weight_fp8):
    """Re-encode jax e4m3fn bytes as IEEE-e4m3 bytes of value/2 (exact for
    normals), transpose to [d, o], and block to [ob, p, kt, obw] so each
    o-block DMA reads 2KB-contiguous per-partition lines."""
    bits = np.arange(256, dtype=np.uint8)
    vals = bits.view(ml_dtypes.float8_e4m3fn).astype(np.float32) * 0.5
    lut = vals.astype(ml_dtypes.float8_e4m3).view(np.uint8)

    wb = np.asarray(weight_fp8).view(np.uint8)          # [DO, DI]
    w2t = np.ascontiguousarray(lut[wb].T)               # [DI, DO]
    w_pre = np.ascontiguousarray(
        w2t.reshape(KT, P, OB, OBW).transpose(2, 1, 0, 3)
    )                                                   # [OB, P, KT, OBW]
    return w_pre.view(ml_dtypes.float8_e4m3)


def kernel(x, weight_fp8, weight_inv_scale, bias):
    from concourse.bass_utils import run_bass_kernel_spmd

    try:
        import jax
        jax.config.update("jax_compilation_cache_dir", "/tmp/jax_neff_cache")
        jax.config.update("jax_persistent_cache_min_entry_size_bytes", 0)
        jax.config.update("jax_persistent_cache_min_compile_time_secs", 0.0)
    except Exception:
        pass

    nc = _get_program()

    x_np = np.asarray(x, dtype=np.float32).reshape(M, DI)
    w_pre = _prep_weights(weight_fp8)
    s_b = np.ascontiguousarray(
        np.broadcast_to(
            np.asarray(weight_inv_scale, dtype=np.float32).reshape(1, 1), (P, 1)
        )
    )
    b_b = np.ascontiguousarray(
        np.broadcast_to(np.asarray(bias, dtype=np.float32), (P, DO))
    )

    core_ids = list(range(NCORES))
    in_maps = [
        {"x": x_np[c * MC:(c + 1) * MC], "w": w_pre, "s": s_b, "b": b_b}
        for c in core_ids
    ]

    last_err = None
    for _attempt in range(3):
        try:
            res = run_bass_kernel_spmd(nc, in_maps, core_ids)
            break
        except Exception as e:  # device wedge (NRT_EXEC_UNIT_UNRECOVERABLE): reset + retry
            last_err = e
            try:
                import jax
                import time
                jax.clear_backends()
                time.sleep(3.0)
            except Exception:
                pass
    else:
        raise last_err

    y = np.concatenate([res.results[c]["y"] for c in core_ids], axis=0)
    return y.reshape(B, S, DO)
